# revision 39
# baseline (speedup 1.0000x reference)
"""Self-contained Trainium2 kernel for nn_AtLBase_54254026883782.

8-layer transformer (L=8500, D=32) + huge MLP head (272000x1024) + cls.
Strategy: sequence-parallel across 8 cores (1072 padded rows each),
per-layer AllGather of x^T, attention computed in transposed layout
[key_tile_partitions, row_free] with softmax denominator via an
augmented ones-column on u = v @ Wo (Wo folded host-side), exp on
ScalarE (bf16 out), row-sharded bf16 matvec for mlp_W + AllReduce.
"""

import math
import numpy as np

D = 32
DFF = 64

FULL = dict(L=8500, NL=8, NOUT=1024, NCLS=230, LPAD=8576, WG=44)
SMALL = dict(L=300, NL=2, NOUT=128, NCLS=16, LPAD=384, WG=4)


def _chunks(total, step):
    return [(s, min(step, total - s)) for s in range(0, total, step)]


def _dims(cfg):
    LP = cfg["LPAD"]
    NC = 8
    R = LP // NC
    RT = (R + 127) // 128
    NT = LP // 128
    NKT = RT * D
    KC = cfg["NOUT"] // 128
    return NC, R, RT, NT, NKT, KC


def _f32(x):
    return np.ascontiguousarray(x, dtype=np.float32)


def prep_inputs(inputs, cfg):
    """Host-side preprocessing: returns in_maps (list of 8 dicts)."""
    import ml_dtypes

    bf = ml_dtypes.bfloat16
    L, NL, NOUT, NCLS, LP = (
        cfg["L"], cfg["NL"], cfg["NOUT"], cfg["NCLS"], cfg["LPAD"])
    NC, R, RT, NT, NKT, KC = _dims(cfg)

    intensity = _f32(inputs["intensity"])[0]
    angle = np.asarray(inputs["angle"])[0].astype(np.int64)
    embed = _f32(inputs["embed"])
    x0 = embed[angle] * intensity[:, None]  # [L, D]
    x0p = np.zeros((LP, D), np.float32)
    x0p[:L] = x0
    ip = np.zeros((LP,), np.float32)
    ip[:L] = intensity

    Wq, bq = _f32(inputs["Wq"]), _f32(inputs["bq"])
    Wk, bk = _f32(inputs["Wk"]), _f32(inputs["bk"])
    Wv, bv = _f32(inputs["Wv"]), _f32(inputs["bv"])
    Wo, bo = _f32(inputs["Wo"]), _f32(inputs["bo"])
    W1, b1 = _f32(inputs["W1"]), _f32(inputs["b1"])
    W2, b2 = _f32(inputs["W2"]), _f32(inputs["b2"])

    sc = 1.0 / math.sqrt(D)

    def aug(W, b):
        return np.concatenate([W, b[:, None, :]], axis=1).astype(np.float32)

    qw = aug(Wq * sc, bq * sc)          # [NL, D+1, D]
    kw = aug(Wk, bk)
    # u = x @ (Wv Wo) + bv Wo, plus ones column for the softmax denominator
    Wvo = np.einsum("lij,ljk->lik", Wv, Wo)
    bvo = np.einsum("lj,ljk->lk", bv, Wo)
    vw = np.zeros((NL, D + 1, D + 1), np.float32)
    vw[:, :D, :D] = Wvo
    vw[:, D, :D] = bvo
    vw[:, D, D] = 1.0
    w1a = aug(W1, b1)                   # [NL, D+1, DFF]
    w2a = aug(W2, b2)                   # [NL, DFF+1, D]

    def repl(v):  # [NL, D] -> [NL, 128, RT*D]
        return np.tile(v[:, None, :], (1, 128, RT)).astype(np.float32)

    g1r = repl(_f32(inputs["ln1_g"]))
    b1r = repl(_f32(inputs["ln1_b"]))
    g2r = repl(_f32(inputs["ln2_g"]))
    b2r = repl(_f32(inputs["ln2_b"]))
    bor = repl(bo)
    ident = np.eye(128, dtype=np.float32)

    mlpW = _f32(inputs["mlp_W"])        # [L*D, NOUT]
    mlp_b = _f32(inputs["mlp_b"])
    clsW = _f32(inputs["cls_W"])        # [NOUT, NCLS]
    cls_b = _f32(inputs["cls_b"])
    Wp = np.zeros((LP, D, NOUT), np.float32)
    Wp[:L] = mlpW.reshape(L, D, NOUT)
    cw = np.zeros((KC + 1, 128, NCLS), np.float32)
    cw[:KC] = clsW.reshape(KC, 128, NCLS)
    cw[KC, 0] = cls_b
    cwb = cw.astype(bf)

    shared = dict(qw=qw.astype(bf), kw=kw.astype(bf), vw=vw.astype(bf),
                  w1=w1a.astype(bf), w2=w2a.astype(bf), g1=g1r, b1=b1r,
                  g2=g2r, b2=b2r, bor=bor, ident=ident, clsw=cwb)
    in_maps = []
    for c in range(NC):
        rows = slice(c * R, (c + 1) * R)
        xT0 = np.zeros((D + 1, R), np.float32)
        xT0[:D] = x0p[rows].T
        xT0[D] = 1.0
        xr = np.zeros((RT * 128, D), np.float32)
        xr[:R] = x0p[rows]
        x0c = np.ascontiguousarray(
            xr.reshape(RT, 128, D).transpose(1, 0, 2).reshape(128, RT * D))
        ir = np.zeros((RT * 128,), np.float32)
        ir[:R] = ip[rows]
        icol = np.ascontiguousarray(ir.reshape(RT, 128).T)
        slab = np.zeros((RT * 128, D, NOUT), np.float32)
        slab[:R] = Wp[rows]
        wre = np.zeros((NKT + 1, 128, NOUT), np.float32)
        wre[:NKT] = slab.reshape(RT, 128, D, NOUT).transpose(
            0, 2, 1, 3).reshape(NKT, 128, NOUT)
        wre[NKT, 0] = mlp_b / NC
        m = dict(shared)
        m.update(xT0=xT0.astype(bf), x0=x0c, icol=icol,
                 wre=wre.astype(bf))
        in_maps.append(m)
    return in_maps


def build_nc(cfg):
    import concourse.bacc as bacc
    import concourse.tile as tile
    from concourse import mybir

    dt = mybir.dt
    F32, BF16, F32R = dt.float32, dt.bfloat16, dt.float32r
    AX = mybir.AxisListType
    OP = mybir.AluOpType
    AF = mybir.ActivationFunctionType

    L, NL, NOUT, NCLS, LP = (
        cfg["L"], cfg["NL"], cfg["NOUT"], cfg["NCLS"], cfg["LPAD"])
    NC, R, RT, NT, NKT, KC = _dims(cfg)
    mlv = 128 - (LP - L)  # valid partitions in last m-tile
    rt_list = [(t * 128, min(128, R - t * 128)) for t in range(RT)]
    passes = _chunks(R, 512)
    WVCOL = cfg.get("WVCOL", True)
    MVCOL = cfg.get("MVCOL", True)
    EPS = 1e-6

    nc = bacc.Bacc("TRN2", target_bir_lowering=False, debug=False,
                   num_devices=NC)

    def din(name, shape, d=F32):
        return nc.dram_tensor(name, list(shape), d, kind="ExternalInput").ap()

    xT0 = din("xT0", [D + 1, R], BF16)
    x0 = din("x0", [128, RT * D])
    icol = din("icol", [128, RT])
    qw = din("qw", [NL, D + 1, D], BF16)
    kw = din("kw", [NL, D + 1, D], BF16)
    vw = din("vw", [NL, D + 1, D + 1], BF16)
    w1 = din("w1", [NL, D + 1, DFF], BF16)
    w2 = din("w2", [NL, DFF + 1, D], BF16)
    g1 = din("g1", [NL, 128, RT * D])
    b1 = din("b1", [NL, 128, RT * D])
    g2 = din("g2", [NL, 128, RT * D])
    b2 = din("b2", [NL, 128, RT * D])
    bor = din("bor", [NL, 128, RT * D])
    ident = din("ident", [128, 128])
    wre = din("wre", [NKT + 1, 128, NOUT], BF16)
    clsw = din("clsw", [KC + 1, 128, NCLS], BF16)
    tap_names = cfg.get("TAPS", [])
    tap_aps = {}
    for tn, tshape, tdt in tap_names:
        tap_aps[tn] = nc.dram_tensor(
            "tap_" + tn, list(tshape), BF16 if tdt == "bf16" else F32,
            kind="ExternalOutput").ap()

    feat_o = nc.dram_tensor("features", [NOUT], F32,
                            kind="ExternalOutput").ap()
    sp_o = nc.dram_tensor("sp", [NCLS], F32, kind="ExternalOutput").ap()
    agin = nc.dram_tensor("agin", [D, R], BF16).ap()
    agout = nc.dram_tensor("agout", [NC, D, R], BF16,
                           addr_space="Shared").ap()
    arin = nc.dram_tensor("arin", [1, NOUT], F32).ap()
    arout = nc.dram_tensor("arout", [1, NOUT], F32,
                           addr_space="Shared").ap()
    RG = [list(range(NC))]

    with tile.TileContext(nc) as tc, \
            tc.tile_pool(name="c1", bufs=1) as cp, \
            tc.tile_pool(name="ln", bufs=2) as lnp, \
            tc.tile_pool(name="eb", bufs=3) as ep, \
            tc.tile_pool(name="wg", bufs=cfg["WG"]) as wp, \
            tc.tile_pool(name="tp", bufs=3) as stp, \
            tc.tile_pool(name="ps_e", bufs=2, space="PSUM") as pseP, \
            tc.tile_pool(name="ps_o", bufs=1, space="PSUM") as psoP, \
            tc.tile_pool(name="ps_m", bufs=2, space="PSUM") as psm:
        sync, vec, ten, gps, sca = (
            nc.sync, nc.vector, nc.tensor, nc.gpsimd, nc.scalar)

        def tap(name, ap):
            if name in tap_aps:
                sync.dma_start(tap_aps[name][...], ap)

        wq_sb = cp.tile([D + 1, NL * D], BF16, tag="wq")
        wk_sb = cp.tile([D + 1, NL * D], BF16, tag="wk")
        vw_sb = cp.tile([D + 1, NL * (D + 1)], BF16, tag="vw")
        w1_sb = cp.tile([D + 1, NL * DFF], BF16, tag="w1")
        w2_sb = cp.tile([DFF + 1, NL * D], BF16, tag="w2")
        for i in range(NL):
            sync.dma_start(wq_sb[:, i * D:(i + 1) * D], qw[i])
            sync.dma_start(wk_sb[:, i * D:(i + 1) * D], kw[i])
            sync.dma_start(vw_sb[:, i * (D + 1):(i + 1) * (D + 1)], vw[i])
            sync.dma_start(w1_sb[:, i * DFF:(i + 1) * DFF], w1[i])
            sync.dma_start(w2_sb[:, i * D:(i + 1) * D], w2[i])
        id_sb = cp.tile([128, 128], F32, tag="id")
        sync.dma_start(id_sb[:, :], ident[:, :])
        id_bf = cp.tile([128, 128], BF16, tag="idb")
        vec.tensor_copy(id_bf[:, :], id_sb[:, :])
        ic_sb = cp.tile([128, RT], F32, tag="ic")
        sync.dma_start(ic_sb[:, :], icol[:, :])
        x_sb = cp.tile([128, RT * D], F32, tag="x")
        sync.dma_start(x_sb[:, :], x0[:, :])
        xT_sb = cp.tile([D + 1, R], BF16, tag="xT")
        sync.dma_start(xT_sb[:, :], xT0[:, :])
        xg_sb = cp.tile([D + 1, LP], BF16, tag="xg")
        gps.memset(xg_sb[D:D + 1, :], 1.0)
        kT_sb = cp.tile([D, LP], BF16, tag="kT")
        qT_sb = cp.tile([D, R], BF16, tag="qT")
        u_sb = cp.tile([128, NT * (D + 1)], BF16, tag="u")
        gps.memset(u_sb[:, :], 0.0)
        uT2 = cp.tile([128, R], F32, tag="uT")
        h_sb = cp.tile([128, RT * D], F32, tag="h")
        gps.memset(h_sb[:, :], 0.0)
        at_sb = cp.tile([128, RT * D], F32, tag="at")
        gps.memset(at_sb[:, :], 0.0)
        ul_sb = cp.tile([128, RT * D], F32, tag="ul")
        gps.memset(ul_sb[:, :], 0.0)
        zc_sb = cp.tile([128, RT * D], F32, tag="zc")
        zq_sb = cp.tile([128, RT * D], F32, tag="zq")
        s1_sb = cp.tile([128, RT], F32, tag="s1")
        s2_sb = cp.tile([128, RT], F32, tag="s2")
        eps_sb = cp.tile([128, 1], F32, tag="eps")
        gps.memset(eps_sb[:, :], EPS)
        x_bf = cp.tile([128, NKT + 1], BF16, tag="xbf")
        gps.memset(x_bf[:, NKT:NKT + 1], 0.0)
        gps.memset(x_bf[0:1, NKT:NKT + 1], 1.0)
        fT32 = cp.tile([128, KC], F32, tag="fT32")
        fT_bf = cp.tile([128, KC + 1], BF16, tag="fTb")
        gps.memset(fT_bf[:, KC:KC + 1], 0.0)
        gps.memset(fT_bf[0:1, KC:KC + 1], 1.0)
        cls_sb = cp.tile([128, (KC + 1) * NCLS], BF16, tag="cls")
        for kt in range(KC + 1):
            sync.dma_start(cls_sb[:, kt * NCLS:(kt + 1) * NCLS], clsw[kt])
        feats_sb = cp.tile([1, NOUT], F32, tag="fs")
        sp_sb = cp.tile([1, NCLS], F32, tag="sps")
        fsum_sb = cp.tile([128, NOUT], F32, tag="fsum")
        gps.memset(fsum_sb[:, :], 0.0)
        sel_sb = cp.tile([128, 1], F32, tag="sel")
        gps.memset(sel_sb[:, :], 0.0)
        for q in range(4):
            gps.memset(sel_sb[32 * q:32 * q + 1, :], 1.0)

        def re3(ap):
            return ap.rearrange("p (t d) -> p t d", d=D)

        def ln_inplace(z, g, b):
            z3, zc3, zq3 = re3(z[:, :]), re3(zc_sb[:, :]), re3(zq_sb[:, :])
            vec.tensor_reduce(s1_sb[:, :], z3, axis=AX.X, op=OP.add)
            vec.tensor_scalar(s1_sb[:, :], s1_sb[:, :], 1.0 / D, None,
                              op0=OP.mult)
            vec.tensor_tensor(zc3, z3,
                              s1_sb[:, :].to_broadcast((128, RT, D)),
                              op=OP.subtract)
            vec.tensor_tensor(zq3, zc3, zc3, op=OP.mult)
            vec.tensor_reduce(s2_sb[:, :], zq3, axis=AX.X, op=OP.add)
            sca.activation(s2_sb[:, :], s2_sb[:, :], AF.Sqrt,
                           bias=eps_sb[:, :], scale=1.0 / D)
            vec.reciprocal(s1_sb[:, :], s2_sb[:, :])
            vec.tensor_tensor(zc3, zc3,
                              s1_sb[:, :].to_broadcast((128, RT, D)),
                              op=OP.mult)
            vec.tensor_tensor(z3, zc3, re3(g[:, :]), op=OP.mult)
            vec.tensor_tensor(z3, z3, re3(b[:, :]), op=OP.add)

        for i in range(NL):
            wqi = wq_sb[:, i * D:(i + 1) * D]
            wki = wk_sb[:, i * D:(i + 1) * D]
            wvi = vw_sb[:, i * (D + 1):(i + 1) * (D + 1)]
            w1i = w1_sb[:, i * DFF:(i + 1) * DFF]
            w2i = w2_sb[:, i * D:(i + 1) * D]
            g1t = lnp.tile([128, RT * D], F32, tag="g1")
            sync.dma_start(g1t[:, :], g1[i])
            b1t = lnp.tile([128, RT * D], F32, tag="b1")
            sync.dma_start(b1t[:, :], b1[i])
            g2t = lnp.tile([128, RT * D], F32, tag="g2")
            sync.dma_start(g2t[:, :], g2[i])
            b2t = lnp.tile([128, RT * D], F32, tag="b2")
            sync.dma_start(b2t[:, :], b2[i])
            bot = lnp.tile([128, RT * D], F32, tag="bo")
            sync.dma_start(bot[:, :], bor[i])

            # all-gather x^T
            sync.dma_start(agin[:, :], xT_sb[0:D, :])
            gps.collective_compute(
                "AllGather", OP.bypass, replica_groups=RG,
                ins=[agin.opt()], outs=[agout.opt()])
            for c in range(NC):
                sync.dma_start(xg_sb[0:D, c * R:(c + 1) * R], agout[c])
            if i == 0:
                tap("xg", xg_sb[0:D, :])

            # kT (global keys, bf16)
            for (s, w) in _chunks(LP, 512):
                pk = psm.tile([D, 512], F32, tag="sm")
                ten.matmul(pk[0:D, 0:w], wki, xg_sb[:, s:s + w],
                           start=True, stop=True)
                vec.tensor_copy(kT_sb[:, s:s + w], pk[0:D, 0:w])
            # qT (local rows, bf16)
            for (s, w) in _chunks(R, 512):
                pq = psm.tile([D, 512], F32, tag="sm")
                ten.matmul(pq[0:D, 0:w], wqi, xT_sb[:, s:s + w],
                           start=True, stop=True)
                vec.tensor_copy(qT_sb[:, s:s + w], pq[0:D, 0:w])
            # u = x @ WvWo + bvWo (+ones col), global, bf16
            for mt in range(NT):
                nv = 128 if mt < NT - 1 else mlv
                pv = psm.tile([128, D + 1], F32, tag="sm")
                ten.matmul(pv[:, :],
                           xg_sb[:, mt * 128:(mt + 1) * 128],
                           wvi, start=True, stop=True)
                vec.tensor_copy(
                    u_sb[0:nv, mt * (D + 1):(mt + 1) * (D + 1)],
                    pv[0:nv, :])
            if i == 0:
                tap("kT", kT_sb[:, :])
                tap("qT", qT_sb[:, :])
                tap("u", u_sb[:, :])

            # attention: scoresT -> exp -> u.T @ e, accumulated over keys.
            # Scores row-packed 2 m-tiles/group into PE strips (kT lives on
            # partition strip mt%4); exp merged per group; wv lags one group
            # so PE fills the exp latency.
            G = cfg.get("G", 2)
            # last m-tile index per column-half (wv accumulation stop flags)
            last_h = [NT - 1 - ((NT - 1 - h) % 2) for h in (0, 1)]
            for (r0, hw) in passes:
                pso = psoP.tile([128, 1024], F32, tag="pso")

                def emit_wv(prev, pso=pso, hw=hw):
                    et_p, p0, pn = prev
                    for j in range(pn):
                        mt = p0 + j
                        if WVCOL:
                            h = mt % 2
                            po = 64 * h
                            ten.matmul(
                                pso[po:po + D + 1, 512 * h:512 * h + hw],
                                u_sb[:, mt * (D + 1):(mt + 1) * (D + 1)],
                                et_p[:, j * hw:(j + 1) * hw],
                                start=(mt == h), stop=(mt == last_h[h]),
                                tile_position=(0, po))
                        else:
                            ten.matmul(
                                pso[0:D + 1, 0:hw],
                                u_sb[:, mt * (D + 1):(mt + 1) * (D + 1)],
                                et_p[:, j * hw:(j + 1) * hw],
                                start=(mt == 0), stop=(mt == NT - 1))

                prev = None
                for g0 in range(0, NT, G):
                    gn = min(G, NT - g0)
                    pse = pseP.tile([128, G * 512], F32, tag="pse")
                    for j in range(gn):
                        mt = g0 + j
                        ten.matmul(pse[:, j * 512:j * 512 + hw],
                                   kT_sb[:, mt * 128:(mt + 1) * 128],
                                   qT_sb[:, r0:r0 + hw],
                                   start=True, stop=True)
                    et = ep.tile([128, G * hw], BF16, tag="e")
                    pse3 = pse[:, 0:gn * 512].rearrange(
                        "p (g c) -> p g c", c=512)[:, :, 0:hw]
                    et3 = et[:, 0:gn * hw].rearrange(
                        "p (g c) -> p g c", c=hw)
                    sca.activation(et3, pse3, AF.Exp)
                    if cfg.get("LAG", True):
                        if prev is not None:
                            emit_wv(prev)
                        prev = (et, g0, gn)
                    else:
                        emit_wv((et, g0, gn))
                if prev is not None:
                    emit_wv(prev)
                vec.tensor_copy(uT2[0:D + 1, r0:r0 + hw],
                                pso[0:D + 1, 0:hw])
                if WVCOL and NT > 1:
                    vec.tensor_copy(uT2[64:64 + D + 1, r0:r0 + hw],
                                    pso[64:64 + D + 1, 512:512 + hw])
                    # merge half B into half A (partition-shifting DMA,
                    # then elementwise add) — offset PE transposes fault
                    ub = stp.tile([D + 1, 512], F32, tag="ub")
                    sync.dma_start(ub[0:D + 1, 0:hw],
                                   uT2[64:64 + D + 1, r0:r0 + hw])
                    vec.tensor_tensor(uT2[0:D + 1, r0:r0 + hw],
                                      uT2[0:D + 1, r0:r0 + hw],
                                      ub[0:D + 1, 0:hw], op=OP.add)


            # epilogue: transpose the two column-halves back (PSUM-
            # accumulated), normalize, +intensity*u_loc
            for t, (t0, sw) in enumerate(rt_list):
                pt = psm.tile([128, D + 1], F32, tag="sm")
                ten.matmul(pt[0:sw, 0:D + 1], uT2[0:D + 1, t0:t0 + sw],
                           id_sb[0:D + 1, 0:D + 1], is_transpose=True,
                           start=True, stop=True)
                rc_t = stp.tile([128, 1], F32, tag="rcp")
                vec.reciprocal(rc_t[0:sw, :], pt[0:sw, D:D + 1])
                vec.tensor_scalar(at_sb[0:sw, t * D:(t + 1) * D],
                                  pt[0:sw, 0:D], rc_t[0:sw, :], None,
                                  op0=OP.mult)
                pu = psm.tile([128, D], F32, tag="sm")
                ten.matmul(pu[0:sw, :],
                           xT_sb[:, t0:t0 + sw],
                           wvi[:, 0:D],
                           start=True, stop=True)
                vec.tensor_scalar(ul_sb[0:sw, t * D:(t + 1) * D],
                                  pu[0:sw, :], ic_sb[0:sw, t:t + 1], None,
                                  op0=OP.mult)
            vec.tensor_tensor(h_sb[:, :], at_sb[:, :], ul_sb[:, :],
                              op=OP.add)
            vec.tensor_tensor(h_sb[:, :], h_sb[:, :], x_sb[:, :], op=OP.add)
            vec.tensor_tensor(h_sb[:, :], h_sb[:, :], bot[:, :], op=OP.add)
            if i == 0:
                tap("at", at_sb[:, :])
                tap("ul", ul_sb[:, :])
                tap("hpre", h_sb[:, :])
            ln_inplace(h_sb, g1t, b1t)
            if i == 0:
                tap("h", h_sb[:, :])

            # MLP
            for t, (t0, sw) in enumerate(rt_list):
                pt1 = psm.tile([D, 128], F32, tag="sm")
                ten.transpose(pt1[0:D, 0:sw], h_sb[0:sw, t * D:(t + 1) * D],
                              id_sb[0:sw, 0:sw])
                hT = stp.tile([D + 1, 128], BF16, tag="hT")
                vec.tensor_copy(hT[0:D, 0:sw], pt1[0:D, 0:sw])
                gps.memset(hT[D:D + 1, 0:sw], 1.0)
                pg = psm.tile([128, DFF], F32, tag="sm")
                ten.matmul(pg[0:sw, :], hT[:, 0:sw],
                           w1i, start=True, stop=True)
                ga = stp.tile([128, DFF], F32, tag="ga")
                gb = stp.tile([128, DFF], BF16, tag="gb")
                vec.tensor_scalar(ga[0:sw, :], pg[0:sw, :], 0.01, None,
                                  op0=OP.mult)
                vec.tensor_tensor(gb[0:sw, :], pg[0:sw, :], ga[0:sw, :],
                                  op=OP.max)
                pt2 = psm.tile([DFF, 128], BF16, tag="sm")
                ten.transpose(pt2[0:DFF, 0:sw], gb[0:sw, :],
                              id_bf[0:sw, 0:sw])
                gT = stp.tile([DFF + 1, 128], BF16, tag="gT")
                vec.tensor_copy(gT[0:DFF, 0:sw], pt2[0:DFF, 0:sw])
                gps.memset(gT[DFF:DFF + 1, 0:sw], 1.0)
                pf = psm.tile([128, D], F32, tag="sm")
                ten.matmul(pf[0:sw, :], gT[:, 0:sw],
                           w2i, start=True, stop=True)
                vec.tensor_tensor(x_sb[0:sw, t * D:(t + 1) * D],
                                  pf[0:sw, :],
                                  h_sb[0:sw, t * D:(t + 1) * D], op=OP.add)
            ln_inplace(x_sb, g2t, b2t)
            if i == 0:
                tap("x1", x_sb[:, :])

            if i < NL - 1:
                for t, (t0, sw) in enumerate(rt_list):
                    pt3 = psm.tile([D, 128], F32, tag="sm")
                    ten.transpose(pt3[0:D, 0:sw],
                                  x_sb[0:sw, t * D:(t + 1) * D],
                                  id_sb[0:sw, 0:sw])
                    vec.tensor_copy(xT_sb[0:D, t0:t0 + sw], pt3[0:D, 0:sw])
                if i == 0:
                    tap("xT1", xT_sb[:, :])

        # final row-sharded matvec over mlp_W, 4x column-tiled (four k-tile
        # stripes accumulate at psum partitions 0/32/64/96, then a selector
        # matmul sums the stripes)
        tap("xfin", x_sb[:, :])
        vec.tensor_copy(x_bf[:, 0:NKT], x_sb[:, :])
        NKT1 = NKT + 1
        fps = []
        for j, (s, w) in enumerate(_chunks(NOUT, 512)):
            pool, tg = (psoP, "pso") if j == 0 else (psm, "sm")
            fpt = pool.tile([128, 512], F32, tag=tg, name="fp%d" % s)
            fps.append((fpt, s, w))
        for kt in range(NKT1):
            wt = wp.tile([128, NOUT], BF16, tag="wt")
            sync.dma_start(wt[:, :], wre[kt])
            q = 32 * (kt % 4) if MVCOL else 0
            for (fpt, s, w) in fps:
                if MVCOL:
                    ten.matmul(fpt[q:q + 1, 0:w], x_bf[:, kt:kt + 1],
                               wt[:, s:s + w], start=(kt < 4),
                               stop=(kt >= NKT1 - 4), tile_position=(0, q))
                else:
                    ten.matmul(fpt[0:1, 0:w], x_bf[:, kt:kt + 1],
                               wt[:, s:s + w], start=(kt == 0),
                               stop=(kt == NKT1 - 1))
        if MVCOL:
            for (fpt, s, w) in fps:
                for q in range(4):
                    vec.tensor_copy(fsum_sb[32 * q:32 * q + 1, s:s + w],
                                    fpt[32 * q:32 * q + 1, 0:w])
            for (fpt, s, w) in fps:
                pfs = psm.tile([1, 512], F32, tag="sm", name="pfs%d" % s)
                ten.matmul(pfs[0:1, 0:w], sel_sb[:, :], fsum_sb[:, s:s + w],
                           start=True, stop=True)
                vec.tensor_copy(feats_sb[0:1, s:s + w], pfs[0:1, 0:w])
        else:
            for (fpt, s, w) in fps:
                vec.tensor_copy(feats_sb[0:1, s:s + w], fpt[0:1, 0:w])
        tap("fpart", feats_sb[0:1, :])
        sync.dma_start(arin[:, :], feats_sb[0:1, :])
        gps.collective_compute(
            "AllReduce", OP.add, replica_groups=RG,
            ins=[arin.opt()], outs=[arout.opt()])
        sync.dma_start(feat_o[:], arout[0, :])
        for kt in range(KC):
            sync.dma_start(fT32[:, kt:kt + 1],
                           arout[0:1, kt * 128:(kt + 1) * 128])
        vec.tensor_copy(fT_bf[:, 0:KC], fT32[:, :])
        psp = psm.tile([1, NCLS], F32, tag="sm")
        for kt in range(KC + 1):
            ten.matmul(psp[0:1, :], fT_bf[:, kt:kt + 1],
                       cls_sb[:, kt * NCLS:(kt + 1) * NCLS],
                       start=(kt == 0), stop=(kt == KC))
        vec.tensor_copy(sp_sb[0:1, :], psp[0:1, :])
        sync.dma_start(sp_o[:], sp_sb[0:1, :])

    nc.compile()
    return nc


_NC_CACHE = {}


def kernel(**inputs):
    import sys
    if "/opt/trn_rl_repo" not in sys.path:
        sys.path.insert(0, "/opt/trn_rl_repo")
    from concourse.bass_utils import run_bass_kernel_spmd

    cfg = FULL
    in_maps = prep_inputs(inputs, cfg)
    key = "full"
    if key not in _NC_CACHE:
        _NC_CACHE[key] = build_nc(cfg)
    nc = _NC_CACHE[key]
    res = run_bass_kernel_spmd(nc, in_maps, list(range(8)))
    feats = np.asarray(res.results[0]["features"],
                       dtype=np.float32).reshape(1, cfg["NOUT"])
    sp = np.asarray(res.results[0]["sp"],
                    dtype=np.float32).reshape(1, cfg["NCLS"])
    return (feats, sp)


# revision 40
# speedup vs baseline: 1.0771x; 1.0771x over previous
"""Self-contained Trainium2 kernel for nn_AtLBase_54254026883782.

8-layer transformer (L=8500, D=32) + huge MLP head (272000x1024) + cls.
Strategy: sequence-parallel across 8 cores (1072 padded rows each),
per-layer AllGather of x^T, attention computed in transposed layout
[key_tile_partitions, row_free] with softmax denominator via an
augmented ones-column on u = v @ Wo (Wo folded host-side), exp on
ScalarE (bf16 out), row-sharded bf16 matvec for mlp_W + AllReduce.
"""

import math
import numpy as np

D = 32
DFF = 64

FULL = dict(L=8500, NL=8, NOUT=1024, NCLS=230, LPAD=8576, WG=44)
SMALL = dict(L=300, NL=2, NOUT=128, NCLS=16, LPAD=384, WG=4)


def _chunks(total, step):
    return [(s, min(step, total - s)) for s in range(0, total, step)]


def _dims(cfg):
    LP = cfg["LPAD"]
    NC = 8
    R = LP // NC
    RT = (R + 127) // 128
    NT = LP // 128
    NKT = RT * D
    KC = cfg["NOUT"] // 128
    return NC, R, RT, NT, NKT, KC


def _f32(x):
    return np.ascontiguousarray(x, dtype=np.float32)


def prep_inputs(inputs, cfg):
    """Host-side preprocessing: returns in_maps (list of 8 dicts)."""
    import ml_dtypes

    bf = ml_dtypes.bfloat16
    L, NL, NOUT, NCLS, LP = (
        cfg["L"], cfg["NL"], cfg["NOUT"], cfg["NCLS"], cfg["LPAD"])
    NC, R, RT, NT, NKT, KC = _dims(cfg)

    intensity = _f32(inputs["intensity"])[0]
    angle = np.asarray(inputs["angle"])[0].astype(np.int64)
    embed = _f32(inputs["embed"])
    x0 = embed[angle] * intensity[:, None]  # [L, D]
    x0p = np.zeros((LP, D), np.float32)
    x0p[:L] = x0
    ip = np.zeros((LP,), np.float32)
    ip[:L] = intensity

    Wq, bq = _f32(inputs["Wq"]), _f32(inputs["bq"])
    Wk, bk = _f32(inputs["Wk"]), _f32(inputs["bk"])
    Wv, bv = _f32(inputs["Wv"]), _f32(inputs["bv"])
    Wo, bo = _f32(inputs["Wo"]), _f32(inputs["bo"])
    W1, b1 = _f32(inputs["W1"]), _f32(inputs["b1"])
    W2, b2 = _f32(inputs["W2"]), _f32(inputs["b2"])

    sc = 1.0 / math.sqrt(D)

    def aug(W, b):
        return np.concatenate([W, b[:, None, :]], axis=1).astype(np.float32)

    qw = aug(Wq * sc, bq * sc)          # [NL, D+1, D]
    kw = aug(Wk, bk)
    # u = x @ (Wv Wo) + bv Wo, plus ones column for the softmax denominator
    Wvo = np.einsum("lij,ljk->lik", Wv, Wo)
    bvo = np.einsum("lj,ljk->lk", bv, Wo)
    vw = np.zeros((NL, D + 1, D + 1), np.float32)
    vw[:, :D, :D] = Wvo
    vw[:, D, :D] = bvo
    vw[:, D, D] = 1.0
    w1a = aug(W1, b1)                   # [NL, D+1, DFF]
    w2a = aug(W2, b2)                   # [NL, DFF+1, D]

    def repl(v):  # [NL, D] -> [NL, 128, RT*D]
        return np.tile(v[:, None, :], (1, 128, RT)).astype(np.float32)

    g1r = repl(_f32(inputs["ln1_g"]))
    b1r = repl(_f32(inputs["ln1_b"]))
    g2r = repl(_f32(inputs["ln2_g"]))
    b2r = repl(_f32(inputs["ln2_b"]))
    bor = repl(bo)
    ident = np.eye(128, dtype=np.float32)

    mlpW = _f32(inputs["mlp_W"])        # [L*D, NOUT]
    mlp_b = _f32(inputs["mlp_b"])
    clsW = _f32(inputs["cls_W"])        # [NOUT, NCLS]
    cls_b = _f32(inputs["cls_b"])
    Wp = np.zeros((LP, D, NOUT), np.float32)
    Wp[:L] = mlpW.reshape(L, D, NOUT)
    cw = np.zeros((KC + 1, 128, NCLS), np.float32)
    cw[:KC] = clsW.reshape(KC, 128, NCLS)
    cw[KC, 0] = cls_b
    cwb = cw.astype(bf)

    shared = dict(qw=qw.astype(bf), kw=kw.astype(bf), vw=vw.astype(bf),
                  w1=w1a.astype(bf), w2=w2a.astype(bf), g1=g1r, b1=b1r,
                  g2=g2r, b2=b2r, bor=bor, ident=ident, clsw=cwb)
    in_maps = []
    for c in range(NC):
        rows = slice(c * R, (c + 1) * R)
        xT0 = np.zeros((D + 1, R), np.float32)
        xT0[:D] = x0p[rows].T
        xT0[D] = 1.0
        xr = np.zeros((RT * 128, D), np.float32)
        xr[:R] = x0p[rows]
        x0c = np.ascontiguousarray(
            xr.reshape(RT, 128, D).transpose(1, 0, 2).reshape(128, RT * D))
        ir = np.zeros((RT * 128,), np.float32)
        ir[:R] = ip[rows]
        icol = np.ascontiguousarray(ir.reshape(RT, 128).T)
        slab = np.zeros((RT * 128, D, NOUT), np.float32)
        slab[:R] = Wp[rows]
        wre = np.zeros((NKT + 1, 128, NOUT), np.float32)
        wre[:NKT] = slab.reshape(RT, 128, D, NOUT).transpose(
            0, 2, 1, 3).reshape(NKT, 128, NOUT)
        wre[NKT, 0] = mlp_b / NC
        m = dict(shared)
        m.update(xT0=xT0.astype(bf), x0=x0c, icol=icol,
                 wre=wre.astype(bf))
        in_maps.append(m)
    return in_maps


def build_nc(cfg):
    import concourse.bacc as bacc
    import concourse.tile as tile
    from concourse import mybir

    dt = mybir.dt
    F32, BF16, F32R = dt.float32, dt.bfloat16, dt.float32r
    AX = mybir.AxisListType
    OP = mybir.AluOpType
    AF = mybir.ActivationFunctionType

    L, NL, NOUT, NCLS, LP = (
        cfg["L"], cfg["NL"], cfg["NOUT"], cfg["NCLS"], cfg["LPAD"])
    NC, R, RT, NT, NKT, KC = _dims(cfg)
    mlv = 128 - (LP - L)  # valid partitions in last m-tile
    rt_list = [(t * 128, min(128, R - t * 128)) for t in range(RT)]
    passes = _chunks(R, 512)
    WVCOL = cfg.get("WVCOL", False)
    LAGN = cfg.get("LAGN", 2)
    MVCOL = cfg.get("MVCOL", False)
    EPS = 1e-6

    nc = bacc.Bacc("TRN2", target_bir_lowering=False, debug=False,
                   num_devices=NC)

    def din(name, shape, d=F32):
        return nc.dram_tensor(name, list(shape), d, kind="ExternalInput").ap()

    xT0 = din("xT0", [D + 1, R], BF16)
    x0 = din("x0", [128, RT * D])
    icol = din("icol", [128, RT])
    qw = din("qw", [NL, D + 1, D], BF16)
    kw = din("kw", [NL, D + 1, D], BF16)
    vw = din("vw", [NL, D + 1, D + 1], BF16)
    w1 = din("w1", [NL, D + 1, DFF], BF16)
    w2 = din("w2", [NL, DFF + 1, D], BF16)
    g1 = din("g1", [NL, 128, RT * D])
    b1 = din("b1", [NL, 128, RT * D])
    g2 = din("g2", [NL, 128, RT * D])
    b2 = din("b2", [NL, 128, RT * D])
    bor = din("bor", [NL, 128, RT * D])
    ident = din("ident", [128, 128])
    wre = din("wre", [NKT + 1, 128, NOUT], BF16)
    clsw = din("clsw", [KC + 1, 128, NCLS], BF16)
    tap_names = cfg.get("TAPS", [])
    tap_aps = {}
    for tn, tshape, tdt in tap_names:
        tap_aps[tn] = nc.dram_tensor(
            "tap_" + tn, list(tshape), BF16 if tdt == "bf16" else F32,
            kind="ExternalOutput").ap()

    feat_o = nc.dram_tensor("features", [NOUT], F32,
                            kind="ExternalOutput").ap()
    sp_o = nc.dram_tensor("sp", [NCLS], F32, kind="ExternalOutput").ap()
    agin = nc.dram_tensor("agin", [D, R], BF16).ap()
    agout = nc.dram_tensor("agout", [NC, D, R], BF16,
                           addr_space="Shared").ap()
    arin = nc.dram_tensor("arin", [1, NOUT], F32).ap()
    arout = nc.dram_tensor("arout", [1, NOUT], F32,
                           addr_space="Shared").ap()
    RG = [list(range(NC))]

    with tile.TileContext(nc) as tc, \
            tc.tile_pool(name="c1", bufs=1) as cp, \
            tc.tile_pool(name="ln", bufs=2) as lnp, \
            tc.tile_pool(name="eb", bufs=cfg.get("EB", 5)) as ep, \
            tc.tile_pool(name="wg", bufs=cfg["WG"]) as wp, \
            tc.tile_pool(name="tp", bufs=3) as stp, \
            tc.tile_pool(name="ps_e", bufs=2, space="PSUM") as pseP, \
            tc.tile_pool(name="ps_o", bufs=1, space="PSUM") as psoP, \
            tc.tile_pool(name="ps_m", bufs=2, space="PSUM") as psm:
        sync, vec, ten, gps, sca = (
            nc.sync, nc.vector, nc.tensor, nc.gpsimd, nc.scalar)

        def tap(name, ap):
            if name in tap_aps:
                sync.dma_start(tap_aps[name][...], ap)

        wq_sb = cp.tile([D + 1, NL * D], BF16, tag="wq")
        wk_sb = cp.tile([D + 1, NL * D], BF16, tag="wk")
        vw_sb = cp.tile([D + 1, NL * (D + 1)], BF16, tag="vw")
        w1_sb = cp.tile([D + 1, NL * DFF], BF16, tag="w1")
        w2_sb = cp.tile([DFF + 1, NL * D], BF16, tag="w2")
        for i in range(NL):
            sync.dma_start(wq_sb[:, i * D:(i + 1) * D], qw[i])
            sync.dma_start(wk_sb[:, i * D:(i + 1) * D], kw[i])
            sync.dma_start(vw_sb[:, i * (D + 1):(i + 1) * (D + 1)], vw[i])
            sync.dma_start(w1_sb[:, i * DFF:(i + 1) * DFF], w1[i])
            sync.dma_start(w2_sb[:, i * D:(i + 1) * D], w2[i])
        id_sb = cp.tile([128, 128], F32, tag="id")
        sync.dma_start(id_sb[:, :], ident[:, :])
        id_bf = cp.tile([128, 128], BF16, tag="idb")
        vec.tensor_copy(id_bf[:, :], id_sb[:, :])
        ic_sb = cp.tile([128, RT], F32, tag="ic")
        sync.dma_start(ic_sb[:, :], icol[:, :])
        x_sb = cp.tile([128, RT * D], F32, tag="x")
        sync.dma_start(x_sb[:, :], x0[:, :])
        xT_sb = cp.tile([D + 1, R], BF16, tag="xT")
        sync.dma_start(xT_sb[:, :], xT0[:, :])
        xg_sb = cp.tile([D + 1, LP], BF16, tag="xg")
        gps.memset(xg_sb[D:D + 1, :], 1.0)
        kT_sb = cp.tile([D, LP], BF16, tag="kT")
        qT_sb = cp.tile([D, R], BF16, tag="qT")
        u_sb = cp.tile([128, NT * (D + 1)], BF16, tag="u")
        gps.memset(u_sb[:, :], 0.0)
        uT2 = cp.tile([128, R], F32, tag="uT")
        h_sb = cp.tile([128, RT * D], F32, tag="h")
        gps.memset(h_sb[:, :], 0.0)
        at_sb = cp.tile([128, RT * D], F32, tag="at")
        gps.memset(at_sb[:, :], 0.0)
        ul_sb = cp.tile([128, RT * D], F32, tag="ul")
        gps.memset(ul_sb[:, :], 0.0)
        zc_sb = cp.tile([128, RT * D], F32, tag="zc")
        zq_sb = cp.tile([128, RT * D], F32, tag="zq")
        s1_sb = cp.tile([128, RT], F32, tag="s1")
        s2_sb = cp.tile([128, RT], F32, tag="s2")
        eps_sb = cp.tile([128, 1], F32, tag="eps")
        gps.memset(eps_sb[:, :], EPS)
        x_bf = cp.tile([128, NKT + 1], BF16, tag="xbf")
        gps.memset(x_bf[:, NKT:NKT + 1], 0.0)
        gps.memset(x_bf[0:1, NKT:NKT + 1], 1.0)
        fT32 = cp.tile([128, KC], F32, tag="fT32")
        fT_bf = cp.tile([128, KC + 1], BF16, tag="fTb")
        gps.memset(fT_bf[:, KC:KC + 1], 0.0)
        gps.memset(fT_bf[0:1, KC:KC + 1], 1.0)
        cls_sb = cp.tile([128, (KC + 1) * NCLS], BF16, tag="cls")
        for kt in range(KC + 1):
            sync.dma_start(cls_sb[:, kt * NCLS:(kt + 1) * NCLS], clsw[kt])
        feats_sb = cp.tile([1, NOUT], F32, tag="fs")
        sp_sb = cp.tile([1, NCLS], F32, tag="sps")
        fsum_sb = cp.tile([128, NOUT], F32, tag="fsum")
        gps.memset(fsum_sb[:, :], 0.0)
        sel_sb = cp.tile([128, 1], F32, tag="sel")
        gps.memset(sel_sb[:, :], 0.0)
        for q in range(4):
            gps.memset(sel_sb[32 * q:32 * q + 1, :], 1.0)

        def re3(ap):
            return ap.rearrange("p (t d) -> p t d", d=D)

        def ln_inplace(z, g, b):
            z3, zc3, zq3 = re3(z[:, :]), re3(zc_sb[:, :]), re3(zq_sb[:, :])
            vec.tensor_reduce(s1_sb[:, :], z3, axis=AX.X, op=OP.add)
            vec.tensor_scalar(s1_sb[:, :], s1_sb[:, :], 1.0 / D, None,
                              op0=OP.mult)
            vec.tensor_tensor(zc3, z3,
                              s1_sb[:, :].to_broadcast((128, RT, D)),
                              op=OP.subtract)
            vec.tensor_tensor(zq3, zc3, zc3, op=OP.mult)
            vec.tensor_reduce(s2_sb[:, :], zq3, axis=AX.X, op=OP.add)
            sca.activation(s2_sb[:, :], s2_sb[:, :], AF.Sqrt,
                           bias=eps_sb[:, :], scale=1.0 / D)
            vec.reciprocal(s1_sb[:, :], s2_sb[:, :])
            vec.tensor_tensor(zc3, zc3,
                              s1_sb[:, :].to_broadcast((128, RT, D)),
                              op=OP.mult)
            vec.tensor_tensor(z3, zc3, re3(g[:, :]), op=OP.mult)
            vec.tensor_tensor(z3, z3, re3(b[:, :]), op=OP.add)

        for i in range(NL):
            wqi = wq_sb[:, i * D:(i + 1) * D]
            wki = wk_sb[:, i * D:(i + 1) * D]
            wvi = vw_sb[:, i * (D + 1):(i + 1) * (D + 1)]
            w1i = w1_sb[:, i * DFF:(i + 1) * DFF]
            w2i = w2_sb[:, i * D:(i + 1) * D]
            g1t = lnp.tile([128, RT * D], F32, tag="g1")
            sync.dma_start(g1t[:, :], g1[i])
            b1t = lnp.tile([128, RT * D], F32, tag="b1")
            sync.dma_start(b1t[:, :], b1[i])
            g2t = lnp.tile([128, RT * D], F32, tag="g2")
            sync.dma_start(g2t[:, :], g2[i])
            b2t = lnp.tile([128, RT * D], F32, tag="b2")
            sync.dma_start(b2t[:, :], b2[i])
            bot = lnp.tile([128, RT * D], F32, tag="bo")
            sync.dma_start(bot[:, :], bor[i])

            # all-gather x^T
            sync.dma_start(agin[:, :], xT_sb[0:D, :])
            gps.collective_compute(
                "AllGather", OP.bypass, replica_groups=RG,
                ins=[agin.opt()], outs=[agout.opt()])
            for c in range(NC):
                sync.dma_start(xg_sb[0:D, c * R:(c + 1) * R], agout[c])
            if i == 0:
                tap("xg", xg_sb[0:D, :])

            # kT (global keys, bf16)
            for (s, w) in _chunks(LP, 512):
                pk = psm.tile([D, 512], F32, tag="sm")
                ten.matmul(pk[0:D, 0:w], wki, xg_sb[:, s:s + w],
                           start=True, stop=True)
                vec.tensor_copy(kT_sb[:, s:s + w], pk[0:D, 0:w])
            # qT (local rows, bf16)
            for (s, w) in _chunks(R, 512):
                pq = psm.tile([D, 512], F32, tag="sm")
                ten.matmul(pq[0:D, 0:w], wqi, xT_sb[:, s:s + w],
                           start=True, stop=True)
                vec.tensor_copy(qT_sb[:, s:s + w], pq[0:D, 0:w])
            # u = x @ WvWo + bvWo (+ones col), global, bf16
            for mt in range(NT):
                nv = 128 if mt < NT - 1 else mlv
                pv = psm.tile([128, D + 1], F32, tag="sm")
                ten.matmul(pv[:, :],
                           xg_sb[:, mt * 128:(mt + 1) * 128],
                           wvi, start=True, stop=True)
                vec.tensor_copy(
                    u_sb[0:nv, mt * (D + 1):(mt + 1) * (D + 1)],
                    pv[0:nv, :])
            if i == 0:
                tap("kT", kT_sb[:, :])
                tap("qT", qT_sb[:, :])
                tap("u", u_sb[:, :])

            # attention: scoresT -> exp -> u.T @ e, accumulated over keys.
            # Scores row-packed 2 m-tiles/group into PE strips (kT lives on
            # partition strip mt%4); exp merged per group; wv lags one group
            # so PE fills the exp latency.
            G = cfg.get("G", 2)
            # last m-tile index per column-half (wv accumulation stop flags)
            last_h = [NT - 1 - ((NT - 1 - h) % 2) for h in (0, 1)]
            for (r0, hw) in passes:
                pso = psoP.tile([128, 1024], F32, tag="pso")

                def emit_wv(prev, pso=pso, hw=hw):
                    et_p, p0, pn = prev
                    for j in range(pn):
                        mt = p0 + j
                        if WVCOL:
                            h = mt % 2
                            po = 64 * h
                            ten.matmul(
                                pso[po:po + D + 1, 512 * h:512 * h + hw],
                                u_sb[:, mt * (D + 1):(mt + 1) * (D + 1)],
                                et_p[:, j * hw:(j + 1) * hw],
                                start=(mt == h), stop=(mt == last_h[h]),
                                tile_position=(0, po))
                        else:
                            ten.matmul(
                                pso[0:D + 1, 0:hw],
                                u_sb[:, mt * (D + 1):(mt + 1) * (D + 1)],
                                et_p[:, j * hw:(j + 1) * hw],
                                start=(mt == 0), stop=(mt == NT - 1))

                prevs = []
                for g0 in range(0, NT, G):
                    gn = min(G, NT - g0)
                    pse = pseP.tile([128, G * 512], F32, tag="pse")
                    for j in range(gn):
                        mt = g0 + j
                        ten.matmul(pse[:, j * 512:j * 512 + hw],
                                   kT_sb[:, mt * 128:(mt + 1) * 128],
                                   qT_sb[:, r0:r0 + hw],
                                   start=True, stop=True)
                    et = ep.tile([128, G * hw], BF16, tag="e")
                    pse3 = pse[:, 0:gn * 512].rearrange(
                        "p (g c) -> p g c", c=512)[:, :, 0:hw]
                    et3 = et[:, 0:gn * hw].rearrange(
                        "p (g c) -> p g c", c=hw)
                    sca.activation(et3, pse3, AF.Exp)
                    if LAGN > 0:
                        prevs.append((et, g0, gn))
                        if len(prevs) > LAGN:
                            emit_wv(prevs.pop(0))
                    else:
                        emit_wv((et, g0, gn))
                for p in prevs:
                    emit_wv(p)
                vec.tensor_copy(uT2[0:D + 1, r0:r0 + hw],
                                pso[0:D + 1, 0:hw])
                if WVCOL and NT > 1:
                    vec.tensor_copy(uT2[64:64 + D + 1, r0:r0 + hw],
                                    pso[64:64 + D + 1, 512:512 + hw])
                    # merge half B into half A (partition-shifting DMA,
                    # then elementwise add) — offset PE transposes fault
                    ub = stp.tile([D + 1, 512], F32, tag="ub")
                    sync.dma_start(ub[0:D + 1, 0:hw],
                                   uT2[64:64 + D + 1, r0:r0 + hw])
                    vec.tensor_tensor(uT2[0:D + 1, r0:r0 + hw],
                                      uT2[0:D + 1, r0:r0 + hw],
                                      ub[0:D + 1, 0:hw], op=OP.add)


            # epilogue: transpose the two column-halves back (PSUM-
            # accumulated), normalize, +intensity*u_loc
            for t, (t0, sw) in enumerate(rt_list):
                pt = psm.tile([128, D + 1], F32, tag="sm")
                ten.matmul(pt[0:sw, 0:D + 1], uT2[0:D + 1, t0:t0 + sw],
                           id_sb[0:D + 1, 0:D + 1], is_transpose=True,
                           start=True, stop=True)
                rc_t = stp.tile([128, 1], F32, tag="rcp")
                vec.reciprocal(rc_t[0:sw, :], pt[0:sw, D:D + 1])
                vec.tensor_scalar(at_sb[0:sw, t * D:(t + 1) * D],
                                  pt[0:sw, 0:D], rc_t[0:sw, :], None,
                                  op0=OP.mult)
                pu = psm.tile([128, D], F32, tag="sm")
                ten.matmul(pu[0:sw, :],
                           xT_sb[:, t0:t0 + sw],
                           wvi[:, 0:D],
                           start=True, stop=True)
                vec.tensor_scalar(ul_sb[0:sw, t * D:(t + 1) * D],
                                  pu[0:sw, :], ic_sb[0:sw, t:t + 1], None,
                                  op0=OP.mult)
            vec.tensor_tensor(h_sb[:, :], at_sb[:, :], ul_sb[:, :],
                              op=OP.add)
            vec.tensor_tensor(h_sb[:, :], h_sb[:, :], x_sb[:, :], op=OP.add)
            vec.tensor_tensor(h_sb[:, :], h_sb[:, :], bot[:, :], op=OP.add)
            if i == 0:
                tap("at", at_sb[:, :])
                tap("ul", ul_sb[:, :])
                tap("hpre", h_sb[:, :])
            ln_inplace(h_sb, g1t, b1t)
            if i == 0:
                tap("h", h_sb[:, :])

            # MLP
            for t, (t0, sw) in enumerate(rt_list):
                pt1 = psm.tile([D, 128], F32, tag="sm")
                ten.transpose(pt1[0:D, 0:sw], h_sb[0:sw, t * D:(t + 1) * D],
                              id_sb[0:sw, 0:sw])
                hT = stp.tile([D + 1, 128], BF16, tag="hT")
                vec.tensor_copy(hT[0:D, 0:sw], pt1[0:D, 0:sw])
                gps.memset(hT[D:D + 1, 0:sw], 1.0)
                pg = psm.tile([128, DFF], F32, tag="sm")
                ten.matmul(pg[0:sw, :], hT[:, 0:sw],
                           w1i, start=True, stop=True)
                ga = stp.tile([128, DFF], F32, tag="ga")
                gb = stp.tile([128, DFF], BF16, tag="gb")
                vec.tensor_scalar(ga[0:sw, :], pg[0:sw, :], 0.01, None,
                                  op0=OP.mult)
                vec.tensor_tensor(gb[0:sw, :], pg[0:sw, :], ga[0:sw, :],
                                  op=OP.max)
                pt2 = psm.tile([DFF, 128], BF16, tag="sm")
                ten.transpose(pt2[0:DFF, 0:sw], gb[0:sw, :],
                              id_bf[0:sw, 0:sw])
                gT = stp.tile([DFF + 1, 128], BF16, tag="gT")
                vec.tensor_copy(gT[0:DFF, 0:sw], pt2[0:DFF, 0:sw])
                gps.memset(gT[DFF:DFF + 1, 0:sw], 1.0)
                pf = psm.tile([128, D], F32, tag="sm")
                ten.matmul(pf[0:sw, :], gT[:, 0:sw],
                           w2i, start=True, stop=True)
                vec.tensor_tensor(x_sb[0:sw, t * D:(t + 1) * D],
                                  pf[0:sw, :],
                                  h_sb[0:sw, t * D:(t + 1) * D], op=OP.add)
            ln_inplace(x_sb, g2t, b2t)
            if i == 0:
                tap("x1", x_sb[:, :])

            if i < NL - 1:
                for t, (t0, sw) in enumerate(rt_list):
                    pt3 = psm.tile([D, 128], F32, tag="sm")
                    ten.transpose(pt3[0:D, 0:sw],
                                  x_sb[0:sw, t * D:(t + 1) * D],
                                  id_sb[0:sw, 0:sw])
                    vec.tensor_copy(xT_sb[0:D, t0:t0 + sw], pt3[0:D, 0:sw])
                if i == 0:
                    tap("xT1", xT_sb[:, :])

        # final row-sharded matvec over mlp_W, 4x column-tiled (four k-tile
        # stripes accumulate at psum partitions 0/32/64/96, then a selector
        # matmul sums the stripes)
        tap("xfin", x_sb[:, :])
        vec.tensor_copy(x_bf[:, 0:NKT], x_sb[:, :])
        NKT1 = NKT + 1
        fps = []
        for j, (s, w) in enumerate(_chunks(NOUT, 512)):
            pool, tg = (psoP, "pso") if j == 0 else (psm, "sm")
            fpt = pool.tile([128, 512], F32, tag=tg, name="fp%d" % s)
            fps.append((fpt, s, w))
        for kt in range(NKT1):
            wt = wp.tile([128, NOUT], BF16, tag="wt")
            sync.dma_start(wt[:, :], wre[kt])
            q = 32 * (kt % 4) if MVCOL else 0
            for (fpt, s, w) in fps:
                if MVCOL:
                    ten.matmul(fpt[q:q + 1, 0:w], x_bf[:, kt:kt + 1],
                               wt[:, s:s + w], start=(kt < 4),
                               stop=(kt >= NKT1 - 4), tile_position=(0, q))
                else:
                    ten.matmul(fpt[0:1, 0:w], x_bf[:, kt:kt + 1],
                               wt[:, s:s + w], start=(kt == 0),
                               stop=(kt == NKT1 - 1))
        if MVCOL:
            for (fpt, s, w) in fps:
                for q in range(4):
                    vec.tensor_copy(fsum_sb[32 * q:32 * q + 1, s:s + w],
                                    fpt[32 * q:32 * q + 1, 0:w])
            for (fpt, s, w) in fps:
                pfs = psm.tile([1, 512], F32, tag="sm", name="pfs%d" % s)
                ten.matmul(pfs[0:1, 0:w], sel_sb[:, :], fsum_sb[:, s:s + w],
                           start=True, stop=True)
                vec.tensor_copy(feats_sb[0:1, s:s + w], pfs[0:1, 0:w])
        else:
            for (fpt, s, w) in fps:
                vec.tensor_copy(feats_sb[0:1, s:s + w], fpt[0:1, 0:w])
        tap("fpart", feats_sb[0:1, :])
        sync.dma_start(arin[:, :], feats_sb[0:1, :])
        gps.collective_compute(
            "AllReduce", OP.add, replica_groups=RG,
            ins=[arin.opt()], outs=[arout.opt()])
        sync.dma_start(feat_o[:], arout[0, :])
        for kt in range(KC):
            sync.dma_start(fT32[:, kt:kt + 1],
                           arout[0:1, kt * 128:(kt + 1) * 128])
        vec.tensor_copy(fT_bf[:, 0:KC], fT32[:, :])
        psp = psm.tile([1, NCLS], F32, tag="sm")
        for kt in range(KC + 1):
            ten.matmul(psp[0:1, :], fT_bf[:, kt:kt + 1],
                       cls_sb[:, kt * NCLS:(kt + 1) * NCLS],
                       start=(kt == 0), stop=(kt == KC))
        vec.tensor_copy(sp_sb[0:1, :], psp[0:1, :])
        sync.dma_start(sp_o[:], sp_sb[0:1, :])

    nc.compile()
    return nc


_NC_CACHE = {}


def kernel(**inputs):
    import sys
    if "/opt/trn_rl_repo" not in sys.path:
        sys.path.insert(0, "/opt/trn_rl_repo")
    from concourse.bass_utils import run_bass_kernel_spmd

    cfg = FULL
    in_maps = prep_inputs(inputs, cfg)
    key = "full"
    if key not in _NC_CACHE:
        _NC_CACHE[key] = build_nc(cfg)
    nc = _NC_CACHE[key]
    res = run_bass_kernel_spmd(nc, in_maps, list(range(8)))
    feats = np.asarray(res.results[0]["features"],
                       dtype=np.float32).reshape(1, cfg["NOUT"])
    sp = np.asarray(res.results[0]["sp"],
                    dtype=np.float32).reshape(1, cfg["NCLS"])
    return (feats, sp)
